# revision 44
# baseline (speedup 1.0000x reference)
"""GAU (Gated Attention Unit) Trainium2 Bass kernel, 8-core sequence-parallel.

Reference computation (all fp32):
    hid  = silu(x @ W_hidden + b_hidden);  v, gate = split(hid, 2)
    qk   = silu(x @ W_qk + b_qk)
    q    = qk * gamma[0] + beta[0];  k = qk * gamma[1] + beta[1]
    attn = relu((q @ k.T) / sqrt(dim))^2
    out  = ((attn @ v) * gate) @ W_out + b_out
    return out * x

Sharding: rows (N=8192) split across 8 cores, 1024 rows each. Each core
computes v / k / q / gate for its OWN rows only, then kT (bf16) and v (fp8)
are AllGathered across the 8 cores (rows land in global core order; the
attention sum over j is order-invariant). The k-gather is issued right
after the qk projection so it overlaps the v compute; the v-gather
overlaps the gate compute and phase-2 sim.

Matmul dtypes:
  - v-projection: fp8 DoubleRow (x and W_v in fp8; W_v staged * 64 to
    escape fp8-subnormal range, compensated via the silu input scale)
  - attn @ v: fp8 DoubleRow, attn scaled by 32 pre-square (compensated
    in W_out on the host); v stored fp8
  - qk / gate / sim / out-projection: bf16 (fp8 fails the 2e-2 gate
    on the gate and output paths)
Measured ~3-4x faster than the replicated-compute baseline; bound by PE
throughput at these dtype rates.
"""

import numpy as np

import concourse.bass as bass
import concourse.mybir as mybir
import concourse.tile as tile
from concourse import bacc

N = 8192          # total rows
D = 1024          # model dim
QK = 200          # qk dim
H = 2048          # hidden (v/gate) dim
NC = 8            # cores
R = N // NC       # rows per core
DT = D // 128     # d-tiles
JGS = N // 512    # j-groups of 512
HT = H // 128     # h-tiles
IC = R // 512     # i-chunks per core

f32 = mybir.dt.float32
f32r = mybir.dt.float32r
bf16 = mybir.dt.bfloat16
f8 = mybir.dt.float8e4
ACT = mybir.ActivationFunctionType
ALU = mybir.AluOpType

# attn is stored as fp8 scaled by FP8_SCALE (applied as sqrt pre-relu-square);
# compensated exactly by scaling W_out on the host by 1/FP8_SCALE.
FP8_SCALE = 32.0
FP8_SQRT = float(np.sqrt(np.float32(FP8_SCALE)))
# v-projection weights are staged as fp8 * VW_SCALE (else they'd be subnormal);
# compensated exactly via the silu activation's input scale.
VW_SCALE = 64.0


def _build_nc(reps=1, vbias=False, obias=False,
              do_p1=True, do_pA=True, do_pB=True, do_pC=True):
    nc = bacc.Bacc("TRN2", target_bir_lowering=False, debug=False)

    xT = nc.dram_tensor("xT", [D, R], bf16, kind="ExternalInput").ap()
    xT8 = nc.dram_tensor("xT8", [D, R], f8, kind="ExternalInput").ap()
    x_own = nc.dram_tensor("x_own", [R, D], f32, kind="ExternalInput").ap()
    w_v8 = nc.dram_tensor("w_v8", [D, H], f8, kind="ExternalInput").ap()
    w_g = nc.dram_tensor("w_g", [D, H], bf16, kind="ExternalInput").ap()
    w_qk = nc.dram_tensor("w_qk", [D, QK], bf16, kind="ExternalInput").ap()
    w_out = nc.dram_tensor("w_out", [H, D], bf16, kind="ExternalInput").ap()
    # per-c scalars, padded 200 -> [2, 128]
    gq = nc.dram_tensor("gq", [2, 128], f32, kind="ExternalInput").ap()
    bq = nc.dram_tensor("bq", [2, 128], f32, kind="ExternalInput").ap()
    gk = nc.dram_tensor("gk", [2, 128], f32, kind="ExternalInput").ap()
    bk = nc.dram_tensor("bk", [2, 128], f32, kind="ExternalInput").ap()
    bqk = nc.dram_tensor("bqk", [2, 128], f32, kind="ExternalInput").ap()
    bg = nc.dram_tensor("bg", [HT, 128], f32, kind="ExternalInput").ap()
    if vbias:
        bv = nc.dram_tensor("bv", [H], f32, kind="ExternalInput").ap()
    if obias:
        bo = nc.dram_tensor("bo", [D], f32, kind="ExternalInput").ap()
    out = nc.dram_tensor("out", [R, D], f32, kind="ExternalOutput").ap()

    with tile.TileContext(nc) as tc:
        with (
            tc.tile_pool(name="pers", bufs=1) as pers,
            tc.tile_pool(name="dram", bufs=1, space="DRAM") as dpool,
        ):
            # persistent small tiles
            gq_t = pers.tile([128, 2], f32)
            bq_t = pers.tile([128, 2], f32)
            gk_t = pers.tile([128, 2], f32)
            bk_t = pers.tile([128, 2], f32)
            bqk_t = pers.tile([128, 2], f32)
            bg_t = pers.tile([128, HT], f32)
            nc.sync.dma_start(out=gq_t, in_=gq.rearrange("ct c -> c ct"))
            nc.sync.dma_start(out=bq_t, in_=bq.rearrange("ct c -> c ct"))
            nc.sync.dma_start(out=gk_t, in_=gk.rearrange("ct c -> c ct"))
            nc.sync.dma_start(out=bk_t, in_=bk.rearrange("ct c -> c ct"))
            nc.sync.dma_start(out=bqk_t, in_=bqk.rearrange("ct c -> c ct"))
            nc.sync.dma_start(out=bg_t, in_=bg.rearrange("ht c -> c ht"))
            if vbias:
                bv_t = pers.tile([128, H], f32)
                nc.sync.dma_start(
                    out=bv_t,
                    in_=bass.AP(tensor=bv.tensor, offset=bv.offset,
                                ap=[[0, 128]] + list(bv.ap)),
                )
            if obias:
                bo_t = pers.tile([128, D], f32)
                nc.sync.dma_start(
                    out=bo_t,
                    in_=bass.AP(tensor=bo.tensor, offset=bo.offset,
                                ap=[[0, 128]] + list(bo.ap)),
                )

            # DRAM scratch: own shards + allgathered full tensors.
            # v is gathered in two row-halves so the first collective can
            # launch halfway through the v compute.
            v_own = dpool.tile([R, H], f8, tag="v_own")
            kT_own = dpool.tile([2, 128, R], bf16, tag="kT_own")
            v_g1 = dpool.tile([NC, R // 2, H], f8, tag="v_g1")
            v_g2 = dpool.tile([NC, R // 2, H], f8, tag="v_g2")
            kT_g = dpool.tile([NC, 2, 128, R], bf16, tag="kT_g")
            # qT / gateT live in SBUF for the whole kernel
            qT_s = pers.tile([128, 2, R], bf16, tag="qT_s", name="qT_s")
            gT_s = pers.tile([128, HT, R], bf16, tag="gT_s", name="gT_s")

            xT_r = xT.rearrange("(dt p) (jg j) -> p dt jg j", p=128, j=512)
            xT8_r = xT8.rearrange("(dt p) (jg j) -> p dt jg j", p=128, j=512)
            wv8_r = w_v8.rearrange("(dt p) h -> p dt h", p=128)
            wg_r = w_g.rearrange("(dt p) h -> p dt h", p=128)
            wqk_r = w_qk.rearrange("(dt p) c -> p dt c", p=128)
            wo_r = w_out.rearrange("(ht p) m -> p ht m", p=128)
            xo_r = x_own.rearrange("(ic it p) m -> p ic it m", p=128, it=4)

            for rep in range(reps):
                if rep:
                    # full barrier between timing reps so SBUF/PSUM region
                    # reuse across the rep boundary is strictly ordered
                    tc.strict_bb_all_engine_barrier()
                # ================= phase 1: v, k, q, gate =================
                if not do_p1:
                    pass
                else:
                 with (
                    tc.tile_pool(name="whp", bufs=1) as whp,
                    tc.tile_pool(name="xgp", bufs=2) as xgp,
                    tc.tile_pool(name="st1", bufs=(2 if vbias else 3)) as st1,
                    tc.tile_pool(name="ps_qk", bufs=2, space="PSUM") as ps_qk,
                    tc.tile_pool(name="ps_v", bufs=2, space="PSUM") as ps_v,
                    tc.tile_pool(name="ps_g", bufs=2, space="PSUM") as ps_g,
                ):
                    wqk_t = whp.tile([128, DT, QK], bf16, tag="wqk")
                    nc.sync.dma_start(out=wqk_t, in_=wqk_r)
                    wv8_t = whp.tile([128, DT, H], f8, tag="wv8")
                    nc.gpsimd.dma_start(out=wv8_t, in_=wv8_r)
                    wg_t = whp.tile([128, DT, H], bf16, tag="wg")
                    for dt in range(DT):
                        eng = nc.sync if dt % 2 else nc.gpsimd
                        eng.dma_start(out=wg_t[:, dt, :], in_=wg_r[:, dt, :])

                    # ---- qk -> k, q (own rows) for both j-groups first,
                    # so the k AllGather launches while v/gate compute ----
                    xgs = []
                    xg8s = []
                    for jg in range(IC):
                        xg = xgp.tile([128, DT, 512], bf16, tag="xg")
                        for dh in range(2):
                            eng = nc.sync if (jg + dh) % 2 else nc.gpsimd
                            eng.dma_start(out=xg[:, dh * 4:(dh + 1) * 4, :],
                                          in_=xT_r[:, dh * 4:(dh + 1) * 4, jg, :])
                        xgs.append(xg)
                        xg8 = xgp.tile([128, DT, 512], f8, tag="xg8")
                        nc.gpsimd.dma_start(out=xg8, in_=xT8_r[:, :, jg, :])
                        xg8s.append(xg8)

                        for ct in range(2):
                            cw = 128 if ct == 0 else QK - 128
                            pq = ps_qk.tile([128, 512], f32)
                            for dt in range(DT):
                                nc.tensor.matmul(
                                    pq[:cw],
                                    wqk_t[:, dt, ct * 128:ct * 128 + cw],
                                    xg[:, dt, :],
                                    start=(dt == 0),
                                    stop=(dt == DT - 1),
                                )
                            sil = st1.tile([128, 512], f32, tag="sil")
                            nc.scalar.activation(
                                sil[:cw], pq[:cw], ACT.Silu,
                                bias=bqk_t[:cw, ct:ct + 1],
                            )
                            kt = st1.tile([128, 512], bf16, tag="kt")
                            nc.vector.tensor_scalar(
                                out=kt[:cw], in0=sil[:cw],
                                scalar1=gk_t[:cw, ct:ct + 1],
                                scalar2=bk_t[:cw, ct:ct + 1],
                                op0=ALU.mult, op1=ALU.add,
                            )
                            nc.sync.dma_start(
                                out=kT_own[ct, 0:cw, jg * 512:(jg + 1) * 512],
                                in_=kt[:cw],
                            )
                            nc.vector.tensor_scalar(
                                out=qT_s[:cw, ct, jg * 512:(jg + 1) * 512],
                                in0=sil[:cw],
                                scalar1=gq_t[:cw, ct:ct + 1],
                                scalar2=bq_t[:cw, ct:ct + 1],
                                op0=ALU.mult, op1=ALU.add,
                            )

                    nc.gpsimd.collective_compute(
                        "AllGather", ALU.bypass,
                        replica_groups=[list(range(NC))],
                        ins=[kT_own.opt()], outs=[kT_g.opt()],
                    )

                    # ---- v (own rows), both j-groups, fp8 DoubleRow ----
                    inv_vw = float(1.0 / VW_SCALE)
                    for jg in range(IC):
                        xg8 = xg8s[jg]
                        for jt in range(4):
                            for hc in range(4):
                                pv = ps_v.tile([128, 512], f32)
                                for tp in range(DT // 2):
                                    nc.tensor.matmul(
                                        pv,
                                        xg8[:, 2 * tp:2 * tp + 2,
                                            jt * 128:(jt + 1) * 128],
                                        wv8_t[:, 2 * tp:2 * tp + 2,
                                              hc * 512:(hc + 1) * 512],
                                        start=(tp == 0),
                                        stop=(tp == DT // 2 - 1),
                                        perf_mode=mybir.MatmulPerfMode.DoubleRow,
                                    )
                                vt = st1.tile([128, 512], f8, tag="vt")
                                if vbias:
                                    tmp = st1.tile([128, 512], f32, tag="vtmp")
                                    nc.vector.tensor_scalar(
                                        out=tmp, in0=pv, scalar1=inv_vw,
                                        op0=ALU.mult)
                                    nc.vector.tensor_add(
                                        tmp, tmp, bv_t[:, hc * 512:(hc + 1) * 512])
                                    nc.scalar.activation(vt, tmp, ACT.Silu)
                                else:
                                    nc.scalar.activation(vt, pv, ACT.Silu,
                                                         scale=inv_vw)
                                veng = nc.sync if (jt + hc) % 2 else nc.gpsimd
                                veng.dma_start(
                                    out=v_own[(jg * 4 + jt) * 128:
                                              (jg * 4 + jt + 1) * 128,
                                              hc * 512:(hc + 1) * 512],
                                    in_=vt,
                                )
                        # gather this j-group's v rows while the next group
                        # (and the gate below) computes
                        nc.gpsimd.collective_compute(
                            "AllGather", ALU.bypass,
                            replica_groups=[list(range(NC))],
                            ins=[v_own[jg * 512:(jg + 1) * 512, :]],
                            outs=[(v_g1 if jg == 0 else v_g2).opt()],
                        )

                    # ---- gateT (own rows), straight into SBUF ----
                    for jg in range(IC):
                        xg = xgs[jg]
                        for ht in range(HT):
                            pg = ps_g.tile([128, 512], f32)
                            for dt in range(DT):
                                nc.tensor.matmul(
                                    pg,
                                    wg_t[:, dt, ht * 128:(ht + 1) * 128],
                                    xg[:, dt, :],
                                    start=(dt == 0),
                                    stop=(dt == DT - 1),
                                )
                            nc.scalar.activation(
                                gT_s[:, ht, jg * 512:(jg + 1) * 512],
                                pg, ACT.Silu, bias=bg_t[:, ht:ht + 1])

                # ================= phase 2: attention per i-chunk =================
                with (
                    tc.tile_pool(name="p2sb", bufs=1) as p2sb,
                    tc.tile_pool(name="kqp", bufs=2) as kqp,
                    tc.tile_pool(name="vst", bufs=5) as vst,
                    tc.tile_pool(name="gst", bufs=2) as gst,
                    tc.tile_pool(name="wop", bufs=2) as wop,
                    tc.tile_pool(name="xop", bufs=2) as xop,
                    tc.tile_pool(name="ost", bufs=2) as osp,
                    tc.tile_pool(name="p2ps", bufs=1, space="PSUM") as p2ps,
                ):
                    # PSUM layout: tag "sim" = 2 banks (A), tag "acc" = 6 banks
                    # (B out1T h-groups of 6/6/4, C out2 4 i-tiles) -> 8 total,
                    # static, so A/B/C of consecutive i-chunks overlap freely.
                    kT_all = p2sb.tile([128, 2, N], bf16, tag="kT_all",
                                       name="kT_all")
                    for c2 in range(NC):
                        keng = nc.sync if c2 % 2 else nc.gpsimd
                        keng.dma_start(
                            out=kT_all[:, :, c2 * R:(c2 + 1) * R],
                            in_=kT_g[c2].rearrange("ct c j -> c ct j"),
                        )
                    for ic in range(IC):
                        attn = p2sb.tile([128, N // 128, 512], f8,
                                         tag="attn", name="attn")
                        gated = p2sb.tile([128, HT, 512], bf16,
                                          tag="gated", name="gated")

                        # ---- A: attn[j, i-chunk] = relu(k.T q)^2 ----
                        if do_pA:
                            q_sb = qT_s[:, :, ic * 512:(ic + 1) * 512]
                            for jt in range(N // 128):
                                pss = p2ps.tile([128, 512], f32, tag="sim",
                                                bufs=2, name="pss")
                                nc.tensor.matmul(
                                    pss, kT_all[:, 0, jt * 128:(jt + 1) * 128],
                                    q_sb[:, 0, :], start=True, stop=False)
                                nc.tensor.matmul(
                                    pss, kT_all[0:QK - 128, 1,
                                                jt * 128:(jt + 1) * 128],
                                    q_sb[0:QK - 128, 1, :],
                                    start=False, stop=True)
                                rel = kqp.tile([128, 512], f32,
                                               tag="rel", bufs=4,
                                               name="rel")
                                nc.scalar.activation(rel, pss, ACT.Relu,
                                                     scale=FP8_SQRT)
                                nc.vector.tensor_mul(
                                    attn[:, jt, :], rel, rel)

                            if not (do_pB and do_pC):
                                pa = kqp.tile([128, 512], f32, tag="pa",
                                              bufs=1, name="pa")
                                nc.vector.tensor_copy(pa, attn[:, 0, :])
                                nc.sync.dma_start(
                                    out=out.rearrange("(a p) m -> p a m", p=128)
                                    [:, 1 + ic, 0:512], in_=pa)

                        # ---- B: out1T[h, i-chunk] = v-lhsT @ attn; * gateT ----
                        # fp8 DoubleRow over j-pairs; jp order visits the
                        # first-gathered v half before the second.
                        jp_order = [jp for jp in range(N // 256) if jp % 4 < 2] \
                            + [jp for jp in range(N // 256) if jp % 4 >= 2]
                        if do_pB:
                            for h0, nht in ((0, 4), (4, 4), (8, 4), (12, 4)):
                                po = p2ps.tile([128, nht, 512], f32, tag="acc",
                                               name="po")
                                for jx, jp in enumerate(jp_order):
                                    vt = vst.tile([128, 2, nht * 128], f8,
                                                  tag="vt", name="vt")
                                    half = jp % 4
                                    vg = v_g1 if half < 2 else v_g2
                                    r0 = (half % 2) * 256
                                    eng = nc.sync if jx % 2 else nc.gpsimd
                                    eng.dma_start(
                                        out=vt,
                                        in_=vg[jp // 4, r0:r0 + 256,
                                               h0 * 128:(h0 + nht) * 128]
                                        .rearrange("(s p) h -> p s h", s=2),
                                    )
                                    for hh in range(nht):
                                        nc.tensor.matmul(
                                            po[:, hh, :],
                                            vt[:, :, hh * 128:(hh + 1) * 128],
                                            attn[:, 2 * jp:2 * jp + 2, :],
                                            start=(jx == 0),
                                            stop=(jx == N // 256 - 1),
                                            perf_mode=mybir.MatmulPerfMode.DoubleRow,
                                        )
                                for hh in range(nht):
                                    ht = h0 + hh
                                    nc.vector.tensor_mul(
                                        gated[:, ht, :], po[:, hh, :],
                                        gT_s[:, ht, ic * 512:(ic + 1) * 512])

                            if not do_pC:
                                pb = gst.tile([128, 512], f32, tag="pb",
                                              bufs=1, name="pb")
                                nc.vector.tensor_copy(pb, gated[:, 0, :])
                                nc.sync.dma_start(
                                    out=out.rearrange("(a p) m -> p a m", p=128)
                                    [:, 4 + ic, 0:512], in_=pb)

                        # ---- C: out2 = gatedT.T @ W_out; out = out2 * x ----
                        # pos uses its own 2-bank PSUM tag so C never
                        # contends with the next i-chunk's B accumulation.
                        if do_pC:
                            for mc in range(2):
                                wo = wop.tile([128, HT, 512], bf16, tag="wo",
                                              name="wo")
                                for hq in range(4):
                                    for dh in range(2):
                                        eng = nc.sync if (hq + dh) % 2 else nc.gpsimd
                                        eng.dma_start(
                                            out=wo[:, hq * 4 + dh * 2:
                                                   hq * 4 + (dh + 1) * 2, :],
                                            in_=wo_r[:, hq * 4 + dh * 2:
                                                     hq * 4 + (dh + 1) * 2,
                                                     mc * 512:(mc + 1) * 512],
                                        )
                                for ith in range(2):
                                    pos = p2ps.tile([128, 2, 512], f32,
                                                    tag="accC", name="pos")
                                    for ht in range(HT):
                                        for i2 in range(2):
                                            it = ith * 2 + i2
                                            nc.tensor.matmul(
                                                pos[:, i2, :],
                                                gated[:, ht,
                                                      it * 128:(it + 1) * 128],
                                                wo[:, ht, :],
                                                start=(ht == 0),
                                                stop=(ht == HT - 1),
                                            )
                                    for i2 in range(2):
                                        it = ith * 2 + i2
                                        xo = xop.tile([128, 1024], f32, tag="xo",
                                                      name="xo")
                                        nc.sync.dma_start(
                                            out=xo, in_=xo_r[:, ic, it, :])
                                        ot = osp.tile([128, 512], f32, tag="ot",
                                                      name="ot")
                                        if obias:
                                            nc.vector.tensor_add(
                                                ot, pos[:, i2, :],
                                                bo_t[:, mc * 512:(mc + 1) * 512])
                                            nc.vector.tensor_mul(
                                                ot, ot,
                                                xo[:, mc * 512:(mc + 1) * 512])
                                        else:
                                            nc.vector.tensor_mul(
                                                ot, pos[:, i2, :],
                                                xo[:, mc * 512:(mc + 1) * 512])
                                        nc.sync.dma_start(
                                            out=out.rearrange(
                                                "(ic it p) m -> p ic it m",
                                                p=128, it=4)
                                            [:, ic, it, mc * 512:(mc + 1) * 512],
                                            in_=ot,
                                        )

            # anchor outputs for phase-subset timing builds (prevents DCE)
            if not (do_pA and do_pB and do_pC):
                tc.strict_bb_all_engine_barrier()
                with tc.tile_pool(name="probe", bufs=1) as prp:
                    if do_p1:
                        pt = prp.tile([128, 512], f32)
                        nc.sync.dma_start(
                            out=pt, in_=v_own[0:128, 0:2048].bitcast(f32))
                        nc.sync.dma_start(
                            out=out.rearrange("(a p) m -> p a m", p=128)
                            [:, 0, 0:512], in_=pt)

    nc.compile()
    return nc


# ---------------------------------------------------------------- runner ----

import time as _time

import jax
import jax.numpy as jnp
from jax.sharding import Mesh, NamedSharding, PartitionSpec
from jax.experimental.shard_map import shard_map

from concourse.bass2jax import _bass_exec_p, install_neuronx_cc_hook, partition_id_tensor


class SpmdRunner:
    def __init__(self, nc, n_cores=8):
        install_neuronx_cc_hook()
        self.nc = nc
        self.n_cores = n_cores
        partition_name = nc.partition_id_tensor.name if nc.partition_id_tensor else None
        in_names, out_names, out_avals, zero_outs = [], [], [], []
        for alloc in nc.m.functions[0].allocations:
            if not isinstance(alloc, mybir.MemoryLocationSet):
                continue
            name = alloc.memorylocations[0].name
            if alloc.kind == "ExternalInput":
                if name != partition_name:
                    in_names.append(name)
            elif alloc.kind == "ExternalOutput":
                shape = tuple(alloc.tensor_shape)
                dtype = mybir.dt.np(alloc.dtype)
                out_names.append(name)
                out_avals.append(jax.core.ShapedArray(shape, dtype))
                zero_outs.append(np.zeros(shape, dtype))
        self.in_names, self.out_names = in_names, out_names
        self.out_avals, self.zero_outs = out_avals, zero_outs
        n_params = len(in_names)
        all_names = in_names + out_names
        if partition_name is not None:
            all_names = all_names + [partition_name]

        def _body(*args):
            operands = list(args)
            if partition_name is not None:
                operands.append(partition_id_tensor())
            outs = _bass_exec_p.bind(
                *operands,
                out_avals=tuple(out_avals),
                in_names=tuple(all_names),
                out_names=tuple(out_names),
                lowering_input_output_aliases=(),
                sim_require_finite=True,
                sim_require_nnan=True,
                nc=nc,
            )
            return tuple(outs)

        devices = jax.devices()[:n_cores]
        self.mesh = Mesh(np.asarray(devices), ("core",))
        in_specs = (PartitionSpec("core"),) * (n_params + len(out_names))
        out_specs = (PartitionSpec("core"),) * len(out_names)
        self.sharded = jax.jit(
            shard_map(_body, mesh=self.mesh, in_specs=in_specs,
                      out_specs=out_specs, check_rep=False),
            keep_unused=True,
        )

    def stage_inputs(self, in_maps):
        n = self.n_cores
        concat = [
            np.concatenate([np.asarray(in_maps[c][name]) for c in range(n)], axis=0)
            for name in self.in_names
        ]
        concat += [np.zeros((n * z.shape[0], *z.shape[1:]), z.dtype)
                   for z in self.zero_outs]
        sharding = NamedSharding(self.mesh, PartitionSpec("core"))
        return [jax.device_put(a, sharding) for a in concat]

    def run(self, staged):
        outs = self.sharded(*staged)
        jax.block_until_ready(outs)
        return outs

    def run_numpy(self, staged):
        outs = self.run(staged)
        n = self.n_cores
        return [
            {name: np.asarray(outs[i]).reshape(n, *self.out_avals[i].shape)[c]
             for i, name in enumerate(self.out_names)}
            for c in range(n)
        ]


# ------------------------------------------------------------- host side ----

_CACHE = {}


def _get_runner(reps, vbias, obias):
    key = (reps, vbias, obias)
    if key not in _CACHE:
        nc = _build_nc(reps=reps, vbias=vbias, obias=obias)
        _CACHE[key] = SpmdRunner(nc, NC)
    return _CACHE[key]


def _pad2(v):
    o = np.zeros((2, 128), np.float32)
    o[0] = v[:128]
    o[1, :QK - 128] = v[128:QK]
    return o


def make_in_maps(x, W_hidden, b_hidden, W_qk, b_qk, gamma, beta, W_out, b_out):
    import ml_dtypes
    bfl = ml_dtypes.bfloat16
    f8l = ml_dtypes.float8_e4m3
    x = np.ascontiguousarray(np.asarray(x, np.float32))
    scale = 1.0 / np.sqrt(np.float32(D))
    gq = _pad2(np.asarray(gamma[0], np.float32) * scale)
    bq = _pad2(np.asarray(beta[0], np.float32) * scale)
    gk = _pad2(np.asarray(gamma[1], np.float32))
    bk = _pad2(np.asarray(beta[1], np.float32))
    bqk = _pad2(np.asarray(b_qk, np.float32))
    bg = np.ascontiguousarray(
        np.asarray(b_hidden[H:], np.float32).reshape(HT, 128))
    W_f32 = np.asarray(W_hidden, np.float32)
    # VW_SCALE lifts the tiny v-weights out of fp8-subnormal range; it is
    # compensated exactly by the silu input scale in the kernel
    Wv8 = np.ascontiguousarray(W_f32[:, :H] * np.float32(VW_SCALE)).astype(f8l)
    Wg = np.ascontiguousarray(W_f32[:, H:]).astype(bfl)
    W_qk = np.ascontiguousarray(np.asarray(W_qk, np.float32)).astype(bfl)
    # 1/FP8_SCALE compensates the fp8 attn scaling (exact: power of two)
    W_out = np.ascontiguousarray(
        np.asarray(W_out, np.float32) * np.float32(1.0 / FP8_SCALE)).astype(bfl)
    bv = np.asarray(b_hidden[:H], np.float32)
    bo = np.asarray(b_out, np.float32)
    vbias = bool(np.any(bv))
    obias = bool(np.any(bo))

    xT = np.ascontiguousarray(x.T.astype(bfl))
    xT8 = np.ascontiguousarray(x.T.astype(f8l))
    in_maps = []
    for c in range(NC):
        m = {
            "xT": np.ascontiguousarray(xT[:, c * R:(c + 1) * R]),
            "xT8": np.ascontiguousarray(xT8[:, c * R:(c + 1) * R]),
            "x_own": x[c * R:(c + 1) * R],
            "w_v8": Wv8,
            "w_g": Wg,
            "w_qk": W_qk,
            "w_out": W_out,
            "gq": gq, "bq": bq, "gk": gk, "bk": bk, "bqk": bqk, "bg": bg,
        }
        if vbias:
            m["bv"] = bv
        if obias:
            m["bo"] = bo
        in_maps.append(m)
    return in_maps, vbias, obias


def kernel(x, W_hidden, b_hidden, W_qk, b_qk, gamma, beta, W_out, b_out):
    in_maps, vbias, obias = make_in_maps(
        x, W_hidden, b_hidden, W_qk, b_qk, gamma, beta, W_out, b_out)
    runner = _get_runner(1, vbias, obias)
    staged = runner.stage_inputs(in_maps)
    results = runner.run_numpy(staged)
    return np.concatenate([results[c]["out"] for c in range(NC)], axis=0)



# revision 45
# speedup vs baseline: 2.2713x; 2.2713x over previous
"""GAU (Gated Attention Unit) Trainium2 Bass kernel, 8-core sequence-parallel.

Reference computation (all fp32):
    hid  = silu(x @ W_hidden + b_hidden);  v, gate = split(hid, 2)
    qk   = silu(x @ W_qk + b_qk)
    q    = qk * gamma[0] + beta[0];  k = qk * gamma[1] + beta[1]
    attn = relu((q @ k.T) / sqrt(dim))^2
    out  = ((attn @ v) * gate) @ W_out + b_out
    return out * x

Sharding: rows (N=8192) split across 8 cores, 1024 rows each. Each core
computes v / k / q / gate for its OWN rows only, then kT (bf16) and v (fp8)
are AllGathered across the 8 cores (rows land in global core order; the
attention sum over j is order-invariant). The k-gather is issued right
after the qk projection so it overlaps the v compute; the v-gather
overlaps the gate compute and phase-2 sim.

Matmul dtypes:
  - v-projection: fp8 DoubleRow (x and W_v in fp8; W_v staged * 64 to
    escape fp8-subnormal range, compensated via the silu input scale)
  - attn @ v: fp8 DoubleRow, attn scaled by 32 pre-square (compensated
    in W_out on the host); v stored fp8
  - qk / gate / sim / out-projection: bf16 (fp8 fails the 2e-2 gate
    on the gate and output paths)
Measured ~3-4x faster than the replicated-compute baseline; bound by PE
throughput at these dtype rates.
"""

import numpy as np

import concourse.bass as bass
import concourse.mybir as mybir
import concourse.tile as tile
from concourse import bacc

N = 8192          # total rows
D = 1024          # model dim
QK = 200          # qk dim
H = 2048          # hidden (v/gate) dim
NC = 8            # cores
R = N // NC       # rows per core
DT = D // 128     # d-tiles
JGS = N // 512    # j-groups of 512
HT = H // 128     # h-tiles
IC = R // 512     # i-chunks per core

f32 = mybir.dt.float32
f32r = mybir.dt.float32r
bf16 = mybir.dt.bfloat16
f8 = mybir.dt.float8e4
ACT = mybir.ActivationFunctionType
ALU = mybir.AluOpType

# attn is stored as fp8 scaled by FP8_SCALE (applied as sqrt pre-relu-square);
# compensated exactly by scaling W_out on the host by 1/FP8_SCALE.
FP8_SCALE = 32.0
FP8_SQRT = float(np.sqrt(np.float32(FP8_SCALE)))
# v-projection weights are staged as fp8 * VW_SCALE (else they'd be subnormal);
# compensated exactly via the silu activation's input scale.
VW_SCALE = 64.0


def _build_nc(reps=1, vbias=False, obias=False,
              do_p1=True, do_pA=True, do_pB=True, do_pC=True):
    nc = bacc.Bacc("TRN2", target_bir_lowering=False, debug=False)

    xT = nc.dram_tensor("xT", [D, R], bf16, kind="ExternalInput").ap()
    xT8 = nc.dram_tensor("xT8", [D, R], f8, kind="ExternalInput").ap()
    x_own = nc.dram_tensor("x_own", [R, D], f32, kind="ExternalInput").ap()
    w_v8 = nc.dram_tensor("w_v8", [D, H], f8, kind="ExternalInput").ap()
    w_g = nc.dram_tensor("w_g", [D, H], bf16, kind="ExternalInput").ap()
    w_qk = nc.dram_tensor("w_qk", [D, QK], bf16, kind="ExternalInput").ap()
    w_out = nc.dram_tensor("w_out", [H, D], bf16, kind="ExternalInput").ap()
    # per-c scalars, padded 200 -> [2, 128]
    gq = nc.dram_tensor("gq", [2, 128], f32, kind="ExternalInput").ap()
    bq = nc.dram_tensor("bq", [2, 128], f32, kind="ExternalInput").ap()
    gk = nc.dram_tensor("gk", [2, 128], f32, kind="ExternalInput").ap()
    bk = nc.dram_tensor("bk", [2, 128], f32, kind="ExternalInput").ap()
    bqk = nc.dram_tensor("bqk", [2, 128], f32, kind="ExternalInput").ap()
    bg = nc.dram_tensor("bg", [HT, 128], f32, kind="ExternalInput").ap()
    if vbias:
        bv = nc.dram_tensor("bv", [H], f32, kind="ExternalInput").ap()
    if obias:
        bo = nc.dram_tensor("bo", [D], f32, kind="ExternalInput").ap()
    out = nc.dram_tensor("out", [R, D], f32, kind="ExternalOutput").ap()

    with tile.TileContext(nc) as tc:
        with (
            tc.tile_pool(name="pers", bufs=1) as pers,
            tc.tile_pool(name="dram", bufs=1, space="DRAM") as dpool,
        ):
            # persistent small tiles
            gq_t = pers.tile([128, 2], f32)
            bq_t = pers.tile([128, 2], f32)
            gk_t = pers.tile([128, 2], f32)
            bk_t = pers.tile([128, 2], f32)
            bqk_t = pers.tile([128, 2], f32)
            bg_t = pers.tile([128, HT], f32)
            nc.sync.dma_start(out=gq_t, in_=gq.rearrange("ct c -> c ct"))
            nc.sync.dma_start(out=bq_t, in_=bq.rearrange("ct c -> c ct"))
            nc.sync.dma_start(out=gk_t, in_=gk.rearrange("ct c -> c ct"))
            nc.sync.dma_start(out=bk_t, in_=bk.rearrange("ct c -> c ct"))
            nc.sync.dma_start(out=bqk_t, in_=bqk.rearrange("ct c -> c ct"))
            nc.sync.dma_start(out=bg_t, in_=bg.rearrange("ht c -> c ht"))
            if vbias:
                bv_t = pers.tile([128, H], f32)
                nc.sync.dma_start(
                    out=bv_t,
                    in_=bass.AP(tensor=bv.tensor, offset=bv.offset,
                                ap=[[0, 128]] + list(bv.ap)),
                )
            if obias:
                bo_t = pers.tile([128, D], f32)
                nc.sync.dma_start(
                    out=bo_t,
                    in_=bass.AP(tensor=bo.tensor, offset=bo.offset,
                                ap=[[0, 128]] + list(bo.ap)),
                )

            # DRAM scratch: own shards + allgathered full tensors
            v_own = dpool.tile([R, H], f8, tag="v_own")
            kT_own = dpool.tile([2, 128, R], bf16, tag="kT_own")
            v_g = dpool.tile([NC, R, H], f8, tag="v_g")
            kT_g = dpool.tile([NC, 2, 128, R], bf16, tag="kT_g")
            # qT / gateT live in SBUF for the whole kernel
            qT_s = pers.tile([128, 2, R], bf16, tag="qT_s", name="qT_s")
            gT_s = pers.tile([128, HT, R], bf16, tag="gT_s", name="gT_s")

            xT_r = xT.rearrange("(dt p) (jg j) -> p dt jg j", p=128, j=512)
            xT8_r = xT8.rearrange("(dt p) (jg j) -> p dt jg j", p=128, j=512)
            wv8_r = w_v8.rearrange("(dt p) h -> p dt h", p=128)
            wg_r = w_g.rearrange("(dt p) h -> p dt h", p=128)
            wqk_r = w_qk.rearrange("(dt p) c -> p dt c", p=128)
            wo_r = w_out.rearrange("(ht p) m -> p ht m", p=128)
            xo_r = x_own.rearrange("(ic it p) m -> p ic it m", p=128, it=4)

            for rep in range(reps):
                if rep:
                    # full barrier between timing reps so SBUF/PSUM region
                    # reuse across the rep boundary is strictly ordered
                    tc.strict_bb_all_engine_barrier()
                # ================= phase 1: v, k, q, gate =================
                if not do_p1:
                    pass
                else:
                 with (
                    tc.tile_pool(name="whp", bufs=1) as whp,
                    tc.tile_pool(name="xgp", bufs=2) as xgp,
                    tc.tile_pool(name="st1", bufs=(2 if vbias else 3)) as st1,
                    tc.tile_pool(name="ps_qk", bufs=2, space="PSUM") as ps_qk,
                    tc.tile_pool(name="ps_v", bufs=2, space="PSUM") as ps_v,
                    tc.tile_pool(name="ps_g", bufs=2, space="PSUM") as ps_g,
                ):
                    wqk_t = whp.tile([128, DT, QK], bf16, tag="wqk")
                    nc.sync.dma_start(out=wqk_t, in_=wqk_r)
                    wv8_t = whp.tile([128, DT, H], f8, tag="wv8")
                    nc.gpsimd.dma_start(out=wv8_t, in_=wv8_r)
                    wg_t = whp.tile([128, DT, H], bf16, tag="wg")
                    for dt in range(DT):
                        eng = nc.sync if dt % 2 else nc.gpsimd
                        eng.dma_start(out=wg_t[:, dt, :], in_=wg_r[:, dt, :])

                    # ---- qk -> k, q (own rows) for both j-groups first,
                    # so the k AllGather launches while v/gate compute ----
                    xgs = []
                    xg8s = []
                    for jg in range(IC):
                        xg = xgp.tile([128, DT, 512], bf16, tag="xg")
                        for dh in range(2):
                            eng = nc.sync if (jg + dh) % 2 else nc.gpsimd
                            eng.dma_start(out=xg[:, dh * 4:(dh + 1) * 4, :],
                                          in_=xT_r[:, dh * 4:(dh + 1) * 4, jg, :])
                        xgs.append(xg)
                        xg8 = xgp.tile([128, DT, 512], f8, tag="xg8")
                        nc.gpsimd.dma_start(out=xg8, in_=xT8_r[:, :, jg, :])
                        xg8s.append(xg8)

                        for ct in range(2):
                            cw = 128 if ct == 0 else QK - 128
                            pq = ps_qk.tile([128, 512], f32)
                            for dt in range(DT):
                                nc.tensor.matmul(
                                    pq[:cw],
                                    wqk_t[:, dt, ct * 128:ct * 128 + cw],
                                    xg[:, dt, :],
                                    start=(dt == 0),
                                    stop=(dt == DT - 1),
                                )
                            sil = st1.tile([128, 512], f32, tag="sil")
                            nc.scalar.activation(
                                sil[:cw], pq[:cw], ACT.Silu,
                                bias=bqk_t[:cw, ct:ct + 1],
                            )
                            kt = st1.tile([128, 512], bf16, tag="kt")
                            nc.vector.tensor_scalar(
                                out=kt[:cw], in0=sil[:cw],
                                scalar1=gk_t[:cw, ct:ct + 1],
                                scalar2=bk_t[:cw, ct:ct + 1],
                                op0=ALU.mult, op1=ALU.add,
                            )
                            nc.sync.dma_start(
                                out=kT_own[ct, 0:cw, jg * 512:(jg + 1) * 512],
                                in_=kt[:cw],
                            )
                            nc.vector.tensor_scalar(
                                out=qT_s[:cw, ct, jg * 512:(jg + 1) * 512],
                                in0=sil[:cw],
                                scalar1=gq_t[:cw, ct:ct + 1],
                                scalar2=bq_t[:cw, ct:ct + 1],
                                op0=ALU.mult, op1=ALU.add,
                            )

                    nc.gpsimd.collective_compute(
                        "AllGather", ALU.bypass,
                        replica_groups=[list(range(NC))],
                        ins=[kT_own.opt()], outs=[kT_g.opt()],
                    )

                    # ---- v (own rows), both j-groups, fp8 DoubleRow ----
                    inv_vw = float(1.0 / VW_SCALE)
                    for jg in range(IC):
                        xg8 = xg8s[jg]
                        for jt in range(4):
                            for hc in range(4):
                                pv = ps_v.tile([128, 512], f32)
                                for tp in range(DT // 2):
                                    nc.tensor.matmul(
                                        pv,
                                        xg8[:, 2 * tp:2 * tp + 2,
                                            jt * 128:(jt + 1) * 128],
                                        wv8_t[:, 2 * tp:2 * tp + 2,
                                              hc * 512:(hc + 1) * 512],
                                        start=(tp == 0),
                                        stop=(tp == DT // 2 - 1),
                                        perf_mode=mybir.MatmulPerfMode.DoubleRow,
                                    )
                                vt = st1.tile([128, 512], f8, tag="vt")
                                if vbias:
                                    tmp = st1.tile([128, 512], f32, tag="vtmp")
                                    nc.vector.tensor_scalar(
                                        out=tmp, in0=pv, scalar1=inv_vw,
                                        op0=ALU.mult)
                                    nc.vector.tensor_add(
                                        tmp, tmp, bv_t[:, hc * 512:(hc + 1) * 512])
                                    nc.scalar.activation(vt, tmp, ACT.Silu)
                                else:
                                    nc.scalar.activation(vt, pv, ACT.Silu,
                                                         scale=inv_vw)
                                veng = nc.sync if (jt + hc) % 2 else nc.gpsimd
                                veng.dma_start(
                                    out=v_own[(jg * 4 + jt) * 128:
                                              (jg * 4 + jt + 1) * 128,
                                              hc * 512:(hc + 1) * 512],
                                    in_=vt,
                                )

                    # v gather launches while gate computes below
                    nc.gpsimd.collective_compute(
                        "AllGather", ALU.bypass,
                        replica_groups=[list(range(NC))],
                        ins=[v_own.opt()], outs=[v_g.opt()],
                    )

                    # ---- gateT (own rows), straight into SBUF ----
                    for jg in range(IC):
                        xg = xgs[jg]
                        for ht in range(HT):
                            pg = ps_g.tile([128, 512], f32)
                            for dt in range(DT):
                                nc.tensor.matmul(
                                    pg,
                                    wg_t[:, dt, ht * 128:(ht + 1) * 128],
                                    xg[:, dt, :],
                                    start=(dt == 0),
                                    stop=(dt == DT - 1),
                                )
                            nc.scalar.activation(
                                gT_s[:, ht, jg * 512:(jg + 1) * 512],
                                pg, ACT.Silu, bias=bg_t[:, ht:ht + 1])

                # ================= phase 2: attention per i-chunk =================
                with (
                    tc.tile_pool(name="p2sb", bufs=1) as p2sb,
                    tc.tile_pool(name="kqp", bufs=2) as kqp,
                    tc.tile_pool(name="vst", bufs=5) as vst,
                    tc.tile_pool(name="gst", bufs=2) as gst,
                    tc.tile_pool(name="wop", bufs=2) as wop,
                    tc.tile_pool(name="xop", bufs=1) as xop,
                    tc.tile_pool(name="ost", bufs=1) as osp,
                    tc.tile_pool(name="p2ps", bufs=1, space="PSUM") as p2ps,
                ):
                    # PSUM layout: tag "sim" = 2 banks (A), tag "acc" = 6 banks
                    # (B out1T h-groups of 6/6/4, C out2 4 i-tiles) -> 8 total,
                    # static, so A/B/C of consecutive i-chunks overlap freely.
                    kT_all = p2sb.tile([128, 2, N], bf16, tag="kT_all",
                                       name="kT_all")
                    for c2 in range(NC):
                        keng = nc.sync if c2 % 2 else nc.gpsimd
                        keng.dma_start(
                            out=kT_all[:, :, c2 * R:(c2 + 1) * R],
                            in_=kT_g[c2].rearrange("ct c j -> c ct j"),
                        )
                    for ic in range(IC):
                        attn = p2sb.tile([128, N // 128, 512], f8,
                                         tag="attn", name="attn")
                        gated = p2sb.tile([128, HT, 512], bf16,
                                          tag="gated", name="gated")

                        # ---- A: attn[j, i-chunk] = relu(k.T q)^2 ----
                        if do_pA:
                            q_sb = qT_s[:, :, ic * 512:(ic + 1) * 512]
                            for jt in range(N // 128):
                                pss = p2ps.tile([128, 512], f32, tag="sim",
                                                bufs=2, name="pss")
                                nc.tensor.matmul(
                                    pss, kT_all[:, 0, jt * 128:(jt + 1) * 128],
                                    q_sb[:, 0, :], start=True, stop=False)
                                nc.tensor.matmul(
                                    pss, kT_all[0:QK - 128, 1,
                                                jt * 128:(jt + 1) * 128],
                                    q_sb[0:QK - 128, 1, :],
                                    start=False, stop=True)
                                rel = kqp.tile([128, 512], f32,
                                               tag="rel", bufs=4,
                                               name="rel")
                                nc.scalar.activation(rel, pss, ACT.Relu,
                                                     scale=FP8_SQRT)
                                nc.vector.tensor_mul(
                                    attn[:, jt, :], rel, rel)

                            if not (do_pB and do_pC):
                                pa = kqp.tile([128, 512], f32, tag="pa",
                                              bufs=1, name="pa")
                                nc.vector.tensor_copy(pa, attn[:, 0, :])
                                nc.sync.dma_start(
                                    out=out.rearrange("(a p) m -> p a m", p=128)
                                    [:, 1 + ic, 0:512], in_=pa)

                        # ---- B: out1T[h, i-chunk] = v-lhsT @ attn; * gateT ----
                        # fp8 DoubleRow: contraction over j-pairs (256 rows
                        # per matmul) at 2x PE rate.
                        if do_pB:
                            for h0, nht in ((0, 6), (6, 6), (12, 4)):
                                po = p2ps.tile([128, nht, 512], f32, tag="acc",
                                               name="po")
                                for jp in range(N // 256):
                                    vt = vst.tile([128, 2, nht * 128], f8,
                                                  tag="vt", name="vt")
                                    eng = nc.sync if jp % 2 else nc.gpsimd
                                    eng.dma_start(
                                        out=vt,
                                        in_=v_g[jp // 4,
                                                (jp % 4) * 256:
                                                (jp % 4 + 1) * 256,
                                                h0 * 128:(h0 + nht) * 128]
                                        .rearrange("(s p) h -> p s h", s=2),
                                    )
                                    for hh in range(nht):
                                        nc.tensor.matmul(
                                            po[:, hh, :],
                                            vt[:, :, hh * 128:(hh + 1) * 128],
                                            attn[:, 2 * jp:2 * jp + 2, :],
                                            start=(jp == 0),
                                            stop=(jp == N // 256 - 1),
                                            perf_mode=mybir.MatmulPerfMode.DoubleRow,
                                        )
                                for hh in range(nht):
                                    ht = h0 + hh
                                    nc.vector.tensor_mul(
                                        gated[:, ht, :], po[:, hh, :],
                                        gT_s[:, ht, ic * 512:(ic + 1) * 512])

                            if not do_pC:
                                pb = gst.tile([128, 512], f32, tag="pb",
                                              bufs=1, name="pb")
                                nc.vector.tensor_copy(pb, gated[:, 0, :])
                                nc.sync.dma_start(
                                    out=out.rearrange("(a p) m -> p a m", p=128)
                                    [:, 4 + ic, 0:512], in_=pb)

                        # ---- C: out2 = gatedT.T @ W_out; out = out2 * x ----
                        if do_pC:
                            for mc in range(2):
                                pos = p2ps.tile([128, 4, 512], f32, tag="acc",
                                                name="pos")
                                for hq in range(4):
                                    wo = wop.tile([128, 4, 512], bf16, tag="wo",
                                                  name="wo")
                                    for dh in range(2):
                                        eng = nc.sync if (hq + dh) % 2 else nc.gpsimd
                                        eng.dma_start(
                                            out=wo[:, dh * 2:(dh + 1) * 2, :],
                                            in_=wo_r[:, hq * 4 + dh * 2:
                                                     hq * 4 + (dh + 1) * 2,
                                                     mc * 512:(mc + 1) * 512],
                                        )
                                    for it in range(4):
                                        for h4 in range(4):
                                            nc.tensor.matmul(
                                                pos[:, it, :],
                                                gated[:, hq * 4 + h4,
                                                      it * 128:(it + 1) * 128],
                                                wo[:, h4, :],
                                                start=(hq == 0 and h4 == 0),
                                                stop=(hq == 3 and h4 == 3),
                                            )
                                for it in range(4):
                                    xo = xop.tile([128, 1024], f32, tag="xo",
                                                  name="xo")
                                    nc.sync.dma_start(
                                        out=xo, in_=xo_r[:, ic, it, :])
                                    ot = osp.tile([128, 512], f32, tag="ot",
                                                  name="ot")
                                    if obias:
                                        nc.vector.tensor_add(
                                            ot, pos[:, it, :],
                                            bo_t[:, mc * 512:(mc + 1) * 512])
                                        nc.vector.tensor_mul(
                                            ot, ot,
                                            xo[:, mc * 512:(mc + 1) * 512])
                                    else:
                                        nc.vector.tensor_mul(
                                            ot, pos[:, it, :],
                                            xo[:, mc * 512:(mc + 1) * 512])
                                    nc.sync.dma_start(
                                        out=out.rearrange(
                                            "(ic it p) m -> p ic it m",
                                            p=128, it=4)
                                        [:, ic, it, mc * 512:(mc + 1) * 512],
                                        in_=ot,
                                    )

            # anchor outputs for phase-subset timing builds (prevents DCE)
            if not (do_pA and do_pB and do_pC):
                tc.strict_bb_all_engine_barrier()
                with tc.tile_pool(name="probe", bufs=1) as prp:
                    if do_p1:
                        pt = prp.tile([128, 512], f32)
                        nc.sync.dma_start(
                            out=pt, in_=v_own[0:128, 0:2048].bitcast(f32))
                        nc.sync.dma_start(
                            out=out.rearrange("(a p) m -> p a m", p=128)
                            [:, 0, 0:512], in_=pt)

    nc.compile()
    return nc


# ---------------------------------------------------------------- runner ----

import time as _time

import jax
import jax.numpy as jnp
from jax.sharding import Mesh, NamedSharding, PartitionSpec
from jax.experimental.shard_map import shard_map

from concourse.bass2jax import _bass_exec_p, install_neuronx_cc_hook, partition_id_tensor


class SpmdRunner:
    def __init__(self, nc, n_cores=8):
        install_neuronx_cc_hook()
        self.nc = nc
        self.n_cores = n_cores
        partition_name = nc.partition_id_tensor.name if nc.partition_id_tensor else None
        in_names, out_names, out_avals, zero_outs = [], [], [], []
        for alloc in nc.m.functions[0].allocations:
            if not isinstance(alloc, mybir.MemoryLocationSet):
                continue
            name = alloc.memorylocations[0].name
            if alloc.kind == "ExternalInput":
                if name != partition_name:
                    in_names.append(name)
            elif alloc.kind == "ExternalOutput":
                shape = tuple(alloc.tensor_shape)
                dtype = mybir.dt.np(alloc.dtype)
                out_names.append(name)
                out_avals.append(jax.core.ShapedArray(shape, dtype))
                zero_outs.append(np.zeros(shape, dtype))
        self.in_names, self.out_names = in_names, out_names
        self.out_avals, self.zero_outs = out_avals, zero_outs
        n_params = len(in_names)
        all_names = in_names + out_names
        if partition_name is not None:
            all_names = all_names + [partition_name]

        def _body(*args):
            operands = list(args)
            if partition_name is not None:
                operands.append(partition_id_tensor())
            outs = _bass_exec_p.bind(
                *operands,
                out_avals=tuple(out_avals),
                in_names=tuple(all_names),
                out_names=tuple(out_names),
                lowering_input_output_aliases=(),
                sim_require_finite=True,
                sim_require_nnan=True,
                nc=nc,
            )
            return tuple(outs)

        devices = jax.devices()[:n_cores]
        self.mesh = Mesh(np.asarray(devices), ("core",))
        in_specs = (PartitionSpec("core"),) * (n_params + len(out_names))
        out_specs = (PartitionSpec("core"),) * len(out_names)
        self.sharded = jax.jit(
            shard_map(_body, mesh=self.mesh, in_specs=in_specs,
                      out_specs=out_specs, check_rep=False),
            keep_unused=True,
        )

    def stage_inputs(self, in_maps):
        n = self.n_cores
        concat = [
            np.concatenate([np.asarray(in_maps[c][name]) for c in range(n)], axis=0)
            for name in self.in_names
        ]
        concat += [np.zeros((n * z.shape[0], *z.shape[1:]), z.dtype)
                   for z in self.zero_outs]
        sharding = NamedSharding(self.mesh, PartitionSpec("core"))
        return [jax.device_put(a, sharding) for a in concat]

    def run(self, staged):
        outs = self.sharded(*staged)
        jax.block_until_ready(outs)
        return outs

    def run_numpy(self, staged):
        outs = self.run(staged)
        n = self.n_cores
        return [
            {name: np.asarray(outs[i]).reshape(n, *self.out_avals[i].shape)[c]
             for i, name in enumerate(self.out_names)}
            for c in range(n)
        ]


# ------------------------------------------------------------- host side ----

_CACHE = {}


def _get_runner(reps, vbias, obias):
    key = (reps, vbias, obias)
    if key not in _CACHE:
        nc = _build_nc(reps=reps, vbias=vbias, obias=obias)
        _CACHE[key] = SpmdRunner(nc, NC)
    return _CACHE[key]


def _pad2(v):
    o = np.zeros((2, 128), np.float32)
    o[0] = v[:128]
    o[1, :QK - 128] = v[128:QK]
    return o


def make_in_maps(x, W_hidden, b_hidden, W_qk, b_qk, gamma, beta, W_out, b_out):
    import ml_dtypes
    bfl = ml_dtypes.bfloat16
    f8l = ml_dtypes.float8_e4m3
    x = np.ascontiguousarray(np.asarray(x, np.float32))
    scale = 1.0 / np.sqrt(np.float32(D))
    gq = _pad2(np.asarray(gamma[0], np.float32) * scale)
    bq = _pad2(np.asarray(beta[0], np.float32) * scale)
    gk = _pad2(np.asarray(gamma[1], np.float32))
    bk = _pad2(np.asarray(beta[1], np.float32))
    bqk = _pad2(np.asarray(b_qk, np.float32))
    bg = np.ascontiguousarray(
        np.asarray(b_hidden[H:], np.float32).reshape(HT, 128))
    W_f32 = np.asarray(W_hidden, np.float32)
    # VW_SCALE lifts the tiny v-weights out of fp8-subnormal range; it is
    # compensated exactly by the silu input scale in the kernel
    Wv8 = np.ascontiguousarray(W_f32[:, :H] * np.float32(VW_SCALE)).astype(f8l)
    Wg = np.ascontiguousarray(W_f32[:, H:]).astype(bfl)
    W_qk = np.ascontiguousarray(np.asarray(W_qk, np.float32)).astype(bfl)
    # 1/FP8_SCALE compensates the fp8 attn scaling (exact: power of two)
    W_out = np.ascontiguousarray(
        np.asarray(W_out, np.float32) * np.float32(1.0 / FP8_SCALE)).astype(bfl)
    bv = np.asarray(b_hidden[:H], np.float32)
    bo = np.asarray(b_out, np.float32)
    vbias = bool(np.any(bv))
    obias = bool(np.any(bo))

    xT = np.ascontiguousarray(x.T.astype(bfl))
    xT8 = np.ascontiguousarray(x.T.astype(f8l))
    in_maps = []
    for c in range(NC):
        m = {
            "xT": np.ascontiguousarray(xT[:, c * R:(c + 1) * R]),
            "xT8": np.ascontiguousarray(xT8[:, c * R:(c + 1) * R]),
            "x_own": x[c * R:(c + 1) * R],
            "w_v8": Wv8,
            "w_g": Wg,
            "w_qk": W_qk,
            "w_out": W_out,
            "gq": gq, "bq": bq, "gk": gk, "bk": bk, "bqk": bqk, "bg": bg,
        }
        if vbias:
            m["bv"] = bv
        if obias:
            m["bo"] = bo
        in_maps.append(m)
    return in_maps, vbias, obias


def kernel(x, W_hidden, b_hidden, W_qk, b_qk, gamma, beta, W_out, b_out):
    in_maps, vbias, obias = make_in_maps(
        x, W_hidden, b_hidden, W_qk, b_qk, gamma, beta, W_out, b_out)
    runner = _get_runner(1, vbias, obias)
    staged = runner.stage_inputs(in_maps)
    results = runner.run_numpy(staged)
    return np.concatenate([results[c]["out"] for c in range(NC)], axis=0)

